# revision 49
# baseline (speedup 1.0000x reference)
"""Trainium2 Bass kernel for nn_BaseContextAwareModel (4-layer GCN + mean-pool + conv1d head).

Strategy (per the graph-id sharding hint): 240 of the 1920 independent 22-node
frame-graphs per NeuronCore (= 2 clips/core), 5 graphs packed per 110-node
block-diagonal chunk. Host precomputes the GCN-normalized dense adjacency
(D^-1/2 (A+I) D^-1/2, transposed, block-diag packed) and folds x @ W1.

Performance design (sim-guided; 30.6us vs 37.9us baseline):
- fp8e4 shipping for ahat (at 8x scale; relu commutes with positive scale so
  1/8 folds into W2..W4 and the conv weights) and for XW1: halves the
  startup-critical DMA bytes. GCN compute stays bf16 with f32 PSUM.
- Input DMAs split across the SP/HWDGE queue AND the Pool/SWDGE queue (which
  bypasses the serialized HWDGE device), sliced so layer 0/1 start while
  later slices stream; head constants ride last on SP.
- Per GCN layer: mm1 (M1t = H_c^T AhatT_c) col-stacks 4 (l3: 2) chunks at
  32-aligned PSUM partition bases via tile_position, 4 (3) more side-by-side
  in the free dim -> ONE wide PSUM->SBUF eviction per 16 (6) chunks. mm2
  (H' = M1t^T W + b, relu fused into eviction) reads the stacked m1 at row
  tile positions and is emitted in band PAIRS, each band writing its own
  bank of a 2-bank PSUM tile: concurrent row tiles must NEVER share a PSUM
  bank (hardware crashes; col tiles may share). Evictions are balanced
  across ACT/DVE by a running-cost model; software pipelining (per-layer
  mm1/mm2 skew) hides eviction latency.
- Head per clip (emitted for clip 0 mid-layer-3): pooledT via matmul with a
  0/1 pool matrix (1/22 folded into conv weights), conv1d(k=3) as shifted
  lhsT matmuls, then sigmoid/capsule-length fused via
  sigmoid(z)-0.5 = tanh(z/2)/2: ACT Tanh (BN-eval scale/2, shift/2), DVE
  square + grouped reduce, one ACT Sqrt(x/16) over both clips (exactly one
  act-table switch), single merged t-major output DMA.
"""

import os
from contextlib import ExitStack

import numpy as np

import concourse.bass as bass
import concourse.bacc as bacc
import concourse.tile as tile
from concourse import mybir
from concourse.bass_utils import run_bass_kernel_spmd

# ---- problem constants (hardcoded; kernel.py must be self-contained) ----
BS, T, P, G = 16, 120, 22, 1920
NCORES = 8
GPC = G // NCORES          # 240 graphs per core
CPG = 5                    # graphs per 128-partition chunk
CH = CPG * P               # 110 nodes per chunk
NCHUNK = GPC // CPG        # 48 chunks per core
BPC = BS // NCORES         # 2 batch items (clips) per core
KPB = T // CPG             # 24 chunks per clip
C_IN = 14
CHS = [16, 32, 64, 152]
DIMS = [C_IN] + CHS
NCLS, DIM_CAP = 17, 16
C_CONV = DIM_CAP * NCLS    # 272
BN_EPS = 1e-3

# layer-0 DMA slices (ahat arrives in these chunk ranges; front-loaded so
# compute can start early, and aligned so l1's first 16-chunk group is
# covered by the first two slices)
L0G = [(0, 5), (5, 16), (16, 28), (28, 38), (38, 48)]

# constsW column layout (bf16): replicated W2/W3/W4, poolm, ones, biases
CW_W2, CW_W3, CW_W4 = 0, 32, 96
CW_POOL, CW_ONES = 248, 253
CW_B1, CW_B2, CW_B3, CW_B4 = 381, 397, 429, 493
CW_W = 645
# constsH column layout (bf16): wc1, wc2, convb, bn(scale/2, shift/2)
CHD_WC1, CHD_WC2, CHD_CB, CHD_BN = 0, 816, 1632, 1904
CHD_W = 1906

F32 = mybir.dt.float32
BF16 = mybir.dt.bfloat16
FP8 = mybir.dt.float8e4
NPBF16 = np.dtype(mybir.dt.np(BF16))
NPFP8 = np.dtype(mybir.dt.np(FP8))
AHAT_SCALE = 8.0  # ahat shipped as fp8e4 at 8x scale; 1/8 folded into W2..W4/wc


TRACE = os.environ.get("KTRACE", "0") == "1"
LAST = None  # last BassKernelResults, for test harness introspection
LAST_NC = None  # last built bass.Bass module, for cost-model simulation


def _host_prep(x, edge_index, edge_attr, W1):
    """Dense normalized adjacency + per-core packed operands."""
    src = np.asarray(edge_index[0], np.int64)
    dst = np.asarray(edge_index[1], np.int64)
    w = np.asarray(edge_attr[:, 4], np.float32)

    A = np.zeros((G, P, P), np.float32)
    np.add.at(A, (dst // P, dst % P, src % P), w)
    deg = A.sum(axis=2) + 1.0                      # + self-loop weight 1
    dinv = 1.0 / np.sqrt(deg)                      # deg >= 1 always
    Ahat = dinv[:, :, None] * A * dinv[:, None, :]
    ii = np.arange(P)
    Ahat[:, ii, ii] += dinv * dinv                 # self loop: dinv[d]^2
    AhatT = np.ascontiguousarray(Ahat.transpose(0, 2, 1))  # [g, s, d]

    # block-diag pack: (NCORES, CH, NCHUNK*CH); rows = source node in chunk,
    # cols = chunk*CH + dest node in chunk
    bd = np.zeros((NCORES, CH, NCHUNK * CH), np.float32)
    bdv = bd.reshape(NCORES, CH, NCHUNK, CH)
    Ar = AhatT.reshape(NCORES, NCHUNK, CPG, P, P)
    for j in range(CPG):
        bdv[:, j * P:(j + 1) * P, :, j * P:(j + 1) * P] = \
            Ar[:, :, j].transpose(0, 2, 1, 3)
    bd *= AHAT_SCALE

    # layer-1 W folded on host: ship XW1 = x @ W1, packed (8, 110, 48, 16)
    xw = np.asarray(x, np.float32) @ np.asarray(W1, np.float32)
    xr = xw.reshape(NCORES, NCHUNK, CH, CHS[0])
    xp = np.ascontiguousarray(xr.transpose(0, 2, 1, 3))
    return bd.astype(NPFP8), xp.astype(NPFP8)


def _pack_consts(W2, W3, W4, bs, conv_w, conv_b, bn_gamma, bn_beta):
    """constsW [128, CW_W] and constsH [128, CHD_W], both bf16."""
    cw = np.zeros((128, CW_W), np.float32)
    for j in range(4):
        cw[32 * j:32 * j + 16, CW_W2:CW_W2 + 32] = W2 / AHAT_SCALE
        cw[32 * j:32 * j + 32, CW_W3:CW_W3 + 64] = W3 / AHAT_SCALE
    for j in range(2):
        cw[64 * j:64 * j + 64, CW_W4:CW_W4 + 152] = W4 / AHAT_SCALE
    for j in range(CPG):
        cw[j * P:(j + 1) * P, CW_POOL + j] = 1.0
    cw[0, CW_ONES:CW_ONES + 128] = 1.0
    cw[0, CW_B1:CW_B1 + 16] = bs[0] * AHAT_SCALE
    cw[0, CW_B2:CW_B2 + 32] = bs[1] * AHAT_SCALE
    cw[0, CW_B3:CW_B3 + 64] = bs[2] * AHAT_SCALE
    cw[0, CW_B4:CW_B4 + 152] = bs[3] * AHAT_SCALE

    ch = np.zeros((128, CHD_W), np.float32)
    # conv weights (co, ci, k) -> (ci, k*272), with the 1/22 mean-pool factor
    wct = np.asarray(conv_w, np.float32).transpose(1, 2, 0) / (float(P) * AHAT_SCALE)
    ch[:, CHD_WC1:CHD_WC1 + 816] = wct[:128].reshape(128, 816)
    ch[:24, CHD_WC2:CHD_WC2 + 816] = wct[128:].reshape(24, 816)
    ch[0, CHD_CB:CHD_CB + C_CONV] = np.asarray(conv_b, np.float32)
    scale = np.asarray(bn_gamma, np.float32) / np.sqrt(1.0 + BN_EPS)
    ch[:T, CHD_BN] = scale * 0.5
    ch[:T, CHD_BN + 1] = np.asarray(bn_beta, np.float32) * 0.5
    return cw.astype(NPBF16), ch.astype(NPBF16)


def _build(nonzero_b, nonzero_convb):
    """Build the SPMD Bass program (identical on all 8 cores)."""
    nc = bacc.Bacc()
    AF = mybir.ActivationFunctionType

    d_ahat = nc.declare_dram_parameter("ahat", [CH, NCHUNK * CH], FP8, isOutput=False)
    d_x = nc.declare_dram_parameter("xp", [CH, NCHUNK, CHS[0]], FP8, isOutput=False)
    d_cw = nc.declare_dram_parameter("cw", [128, CW_W], BF16, isOutput=False)
    d_ch = nc.declare_dram_parameter("chd", [128, CHD_W], BF16, isOutput=False)
    d_out = nc.declare_dram_parameter("out", [BPC * T, NCLS], F32, isOutput=True)

    with tile.TileContext(nc) as tc, ExitStack() as ctx:
        const = ctx.enter_context(tc.tile_pool(name="const", bufs=1))
        state = ctx.enter_context(tc.tile_pool(name="state", bufs=1))
        m1p = ctx.enter_context(tc.tile_pool(name="m1sb", bufs=3))
        ps = ctx.enter_context(tc.tile_pool(name="ps", bufs=2, space="PSUM"))
        head = ctx.enter_context(tc.tile_pool(name="head", bufs=2))

        # ---- input DMAs ----
        # SP engine (HWDGE): ahat+xw interleaved so layer 0 starts ASAP.
        t_ahat_sl = []
        for i, (a, b) in enumerate(L0G):
            ta = const.tile([CH, (b - a) * CH], FP8, tag=f"ahat{i}", name=f"ta{i}")
            t_ahat_sl.append(ta)
        t_x = const.tile([CH, NCHUNK, CHS[0]], FP8)
        # dispatch is split across two queues (SP/HWDGE and Pool/SWDGE) so
        # the ~0.6-1us per-DMA dispatch overheads run in parallel
        nc.sync.dma_start(out=t_ahat_sl[0], in_=d_ahat[:, 0:5 * CH])
        nc.sync.dma_start(out=t_x[:, 0:16, :], in_=d_x[:, 0:16, :])
        nc.sync.dma_start(out=t_ahat_sl[2], in_=d_ahat[:, 16 * CH:28 * CH])
        nc.sync.dma_start(out=t_x[:, 16:48, :], in_=d_x[:, 16:48, :])
        nc.sync.dma_start(out=t_ahat_sl[4], in_=d_ahat[:, 38 * CH:48 * CH])
        # head constants ride the SP queue last: their transfer must not
        # delay the compute-critical ahat stream
        t_ch = const.tile([128, CHD_W], BF16)
        nc.sync.dma_start(out=t_ch, in_=d_ch[:])
        nc.gpsimd.dma_start(out=t_ahat_sl[1], in_=d_ahat[:, 5 * CH:16 * CH])
        t_cw = const.tile([128, CW_W], BF16)
        nc.gpsimd.dma_start(out=t_cw, in_=d_cw[:])
        nc.gpsimd.dma_start(out=t_ahat_sl[3], in_=d_ahat[:, 28 * CH:38 * CH])

        def ahat_chunk(k):
            for i, (a, b) in enumerate(L0G):
                if a <= k < b:
                    return t_ahat_sl[i][:, (k - a) * CH:(k - a + 1) * CH]
            raise AssertionError(k)

        # eviction engine selection: running-debt balance between ACT
        # (0.833 ns/elem + ~143 ns) and DVE (1.042 ns/elem + ~125 ns)
        debt = {"act": 0.0, "dve": 0.0}  # ACT also pays the tail table load

        def evict(dst, src, relu, eng=None):
            n = src.free_size()
            c_act, c_dve = n * 0.833 + 143.0, n * 1.042 + 125.0
            use_act = (eng == "act") if eng else (
                debt["act"] + c_act <= debt["dve"] + c_dve)
            if use_act:
                debt["act"] += c_act
                nc.scalar.activation(dst, src, AF.Relu if relu else AF.Copy)
            else:
                debt["dve"] += c_dve
                if relu:
                    nc.vector.tensor_scalar_max(dst, src, 0.0)
                else:
                    nc.vector.tensor_copy(dst, src)

        # ---- layer 0: H1 = relu(Ahat @ XW1 + b1), chunk groups = DMA slices
        # h1 carries 16 zero pad channels so l1's mm1 (lhsT free dim 32)
        # fully writes its 32-row PSUM bands (no uninitialized PSUM reads)
        t_ones = t_cw[0:1, CW_ONES:CW_ONES + 128]
        h1 = state.tile([CH, NCHUNK, 32], BF16)
        nc.gpsimd.memset(h1[:, :, CHS[0]:32], 0.0)
        for g, (a, b) in enumerate(L0G):
            ps_h = ps.tile([128, 512], F32, tag="m1", name="ps0", bufs=3)
            for k in range(a, b):
                j = k - a
                nc.tensor.matmul(
                    ps_h[:CH, j * 16:(j + 1) * 16],
                    lhsT=ahat_chunk(k), rhs=t_x[:, k, :],
                    start=True, stop=not nonzero_b[0])
                if nonzero_b[0]:
                    nc.tensor.matmul(
                        ps_h[:CH, j * 16:(j + 1) * 16],
                        lhsT=t_ones[:, :CH], rhs=t_cw[0:1, CW_B1:CW_B1 + 16],
                        start=False, stop=True, skip_group_check=True)
            evict(h1[:, a:b, 0:CHS[0]], ps_h[:CH, :(b - a) * 16], relu=True)

        # ---- head emission (called per clip; clip 0 interleaved into l3) ----
        bn_done = [False]
        t_bn = const.tile([T, 2], F32)
        ssum = head.tile([T, BPC * NCLS], F32, tag="ssum", name="ssum", bufs=1)

        def emit_head(b, h4):
            if not bn_done[0]:
                # bn scale/shift cast to f32 for the ACT bias/scale operands
                # (emitted late so DVE's in-order queue doesn't stall on the
                # chd DMA early on)
                nc.vector.tensor_copy(t_bn, t_ch[0:T, CHD_BN:CHD_BN + 2])
                bn_done[0] = True
            t_pool = t_cw[0:CH, CW_POOL:CW_POOL + CPG]
            # pooledT: pt1 (128ch) at psum cols 0:120, pt2 (24ch) at 128:248
            ps_pt = ps.tile([128, 512], F32, tag="head", name="pspt", bufs=1)
            for kk in range(KPB):
                k = b * KPB + kk
                nc.tensor.matmul(ps_pt[0:128, kk * CPG:(kk + 1) * CPG],
                                 lhsT=h4[:, k, :128], rhs=t_pool,
                                 start=True, stop=True)
                nc.tensor.matmul(ps_pt[0:24, 128 + kk * CPG:128 + (kk + 1) * CPG],
                                 lhsT=h4[:, k, 128:], rhs=t_pool,
                                 start=True, stop=True)
            # pt sbuf: [128, 244] = two 122-col blocks (1-col zero pad each side)
            t_pt = head.tile([128, 244], BF16, tag="pt", name="tpt")
            ptv = t_pt.rearrange("p (b c) -> p b c", b=2)
            nc.gpsimd.memset(ptv[:, :, 0:1], 0.0)
            nc.gpsimd.memset(ptv[:, :, 121:122], 0.0)
            evict(ptv[0:128, 0, 1:121], ps_pt[0:128, 0:120], relu=False)
            evict(ptv[0:24, 1, 1:121], ps_pt[0:24, 128:248], relu=False)

            # conv1d(k=3): 6 accumulating matmuls into one (120, 272) bank
            ps_caps = ps.tile([128, 512], F32, tag="head", name="pscaps", bufs=1)
            nmm = 6 + (1 if nonzero_convb else 0)
            i = 0
            for blk, p0, n in ((0, 0, 128), (1, 0, 24)):
                for kk in range(3):
                    wcols = (CHD_WC1 if blk == 0 else CHD_WC2) + kk * C_CONV
                    nc.tensor.matmul(
                        ps_caps[0:T, 0:C_CONV],
                        lhsT=t_pt[p0:p0 + n, blk * 122 + kk:blk * 122 + kk + T],
                        rhs=t_ch[p0:p0 + n, wcols:wcols + C_CONV],
                        start=(i == 0), stop=(i == nmm - 1),
                        skip_group_check=True)
                    i += 1
            if nonzero_convb:
                nc.tensor.matmul(ps_caps[0:T, 0:C_CONV], lhsT=t_ones[:, :T],
                                 rhs=t_ch[0:1, CHD_CB:CHD_CB + C_CONV],
                                 start=False, stop=True, skip_group_check=True)

            # sigmoid(z)-0.5 = tanh(z/2)/2; capsule length via square+reduce
            th = head.tile([T, C_CONV], BF16, tag="th", name="th")
            nc.scalar.activation(th, ps_caps[0:T, 0:C_CONV], AF.Tanh,
                                 bias=t_bn[:, 1:2], scale=t_bn[:, 0:1])
            debt["act"] += 370.0
            sq = head.tile([T, C_CONV], BF16, tag="sq", name="sq")
            nc.vector.tensor_mul(sq, th, th)
            nc.vector.reduce_sum(
                out=ssum[:, b * NCLS:(b + 1) * NCLS],
                in_=sq.rearrange("p (d c) -> p c d", c=NCLS),
                axis=mybir.AxisListType.X)

        # ---- layers 1..3 ----
        # mm1 col-stacks nband chunks per PSUM bank (tile_position col =
        # band*step) -> one wide m1 eviction per group. mm2 runs at row
        # tile_position band*step and is emitted in band PAIRS, each pair
        # writing its own bank of a 2-bank "hpair" tile: concurrent row
        # tiles must never share a PSUM bank (HW crashes), col tiles may.
        # per-layer: (cin, cin_load, cout, step, nband, wins, W col, b col)
        LSPEC = [
            (16, 32, 32, 32, 4, 4, CW_W2, CW_B2),
            (32, 32, 64, 32, 4, 4, CW_W3, CW_B3),
            (64, 64, 152, 64, 2, 3, CW_W4, CW_B4),
        ]
        h_prev = h1
        prev_tail = []
        for li, (cin, cin_load, cout, step, nband, wins, wcol, bcol) in enumerate(LSPEC):
            l = li + 1
            grp = nband * wins           # chunks per mm1 PSUM bank
            ngrp = NCHUNK // grp
            npair = nband // 2           # mm2 band-pairs per mm1 group
            nb_rows = (nband - 1) * step + cin_load
            h_next = state.tile([CH, NCHUNK, cout], BF16, tag=f"h{l}", name=f"h{l}")

            m1ref = {}

            def mm1_group(g, cin_load=cin_load, step=step, wins=wins,
                          grp=grp, nb_rows=nb_rows):
                ps_m1 = ps.tile([128, 512], F32, tag="m1", name="psm1", bufs=3)
                for j in range(grp):
                    k = g * grp + j
                    band, w = j // wins, j % wins
                    nc.tensor.matmul(
                        ps_m1[band * step:band * step + cin_load, w * CH:(w + 1) * CH],
                        lhsT=h_prev[:, k, :cin_load], rhs=ahat_chunk(k),
                        start=True, stop=True,
                        tile_position=(0, band * step))
                m1_sb = m1p.tile([128, 440], BF16, tag="m1sb", name="m1sb")
                evict(m1_sb[:nb_rows, :wins * CH], ps_m1[:nb_rows, :wins * CH],
                      relu=False)
                for j in range(grp):
                    band, w = j // wins, j % wins
                    m1ref[g * grp + j] = (m1_sb, band * step, w * CH)

            def mm2_pair(g, pr, cin=cin, cout=cout, step=step, wins=wins,
                         grp=grp, wcol=wcol, bcol=bcol, l=l, m1ref=m1ref,
                         h_next=h_next):
                # bands 2*pr and 2*pr+1 -> banks 0 and 1 of this pair tile
                ps_h = ps.tile([128, 1024], F32, tag="hpair", name="psh", bufs=2)
                for jj in range(2 * wins):
                    half, w = jj // wins, jj % wins
                    band = 2 * pr + half
                    sb, pb, co = m1ref[g * grp + band * wins + w]
                    dst_c = half * 512 + w * cout
                    nc.tensor.matmul(
                        ps_h[:CH, dst_c:dst_c + cout],
                        lhsT=sb[pb:pb + cin, co:co + CH],
                        rhs=t_cw[pb:pb + cin, wcol:wcol + cout],
                        start=True, stop=not nonzero_b[l],
                        tile_position=(pb, 0))
                    if nonzero_b[l]:
                        nc.tensor.matmul(
                            ps_h[:CH, dst_c:dst_c + cout],
                            lhsT=t_ones[:, :CH], rhs=t_cw[0:1, bcol:bcol + cout],
                            start=False, stop=True, skip_group_check=True)
                # one eviction: dst chunk order (half, w, c) matches src
                c0 = g * grp + 2 * pr * wins
                dst = h_next[:, c0:c0 + 2 * wins, :]
                src = ps_h[:CH].rearrange("p (b c) -> p b c", b=2)[:, :, :wins * cout]
                evict(dst, src, relu=True)
                return c0 + 2 * wins     # chunks completed so far

            # software pipeline: mm1 emitted ahead of mm2; for the last
            # layer, clip 0's head is emitted as soon as its chunks are done
            last = li == len(LSPEC) - 1

            def mm2_step(state_, h_next=h_next, last=last, npair=npair,
                         mm2_pair=mm2_pair):
                g, pr = state_
                hi = mm2_pair(g, pr)
                if last and hi == KPB:
                    emit_head(0, h_next)
                pr += 1
                return (g + 1, 0) if pr == npair else (g, pr)

            SKEW = int(os.environ.get("KSKEW", "212")[li])
            cur = (0, 0)
            for g in range(ngrp):
                mm1_group(g)
                if g == 0:
                    # finish the previous layer's deferred mm2 pairs here so
                    # they hide behind this layer's first mm1 group
                    for f in prev_tail:
                        f()
                    prev_tail = []
                while cur[0] <= g - SKEW:
                    cur = mm2_step(cur)
            # defer the trailing pairs into the next layer's emission window
            ndef = 0 if last else int(os.environ.get("KDEF", "0"))
            left = []
            while cur[0] < ngrp:
                left.append(cur)
                cur = (cur[0] + 1, 0) if cur[1] + 1 == npair else (cur[0], cur[1] + 1)
            for g_, pr_ in left[:len(left) - ndef]:
                mm2_step((g_, pr_))
            for g_, pr_ in left[len(left) - ndef:]:
                prev_tail.append(lambda g_=g_, pr_=pr_, f=mm2_step: f((g_, pr_)))
            h_prev = h_next

        # ---- head for clip 1 (clip 0 was interleaved into layer 3) ----
        emit_head(1, h_prev)
        # one Sqrt op over both clips -> exactly one act-table switch
        y = head.tile([T, BPC * NCLS], F32, tag="y", name="y", bufs=1)
        nc.scalar.activation(y, ssum, AF.Sqrt, scale=1.0 / DIM_CAP)
        nc.sync.dma_start(
            out=d_out.rearrange("(b t) c -> t b c", b=BPC),
            in_=y.rearrange("p (b c) -> p b c", b=BPC))

    return nc


def kernel(x, edge_index, batch, edge_attr, W1, b1, W2, b2, W3, b3, W4, b4,
           conv_w, conv_b, bn_gamma, bn_beta):
    global LAST, LAST_NC
    bd, xp = _host_prep(x, edge_index, edge_attr, W1)

    bs = [np.asarray(b_, np.float32) for b_ in (b1, b2, b3, b4)]
    nonzero_b = [bool(np.any(b_)) for b_ in bs]
    nonzero_convb = bool(np.any(np.asarray(conv_b, np.float32)))
    cw, chd = _pack_consts(np.asarray(W2, np.float32), np.asarray(W3, np.float32),
                           np.asarray(W4, np.float32), bs, conv_w, conv_b,
                           bn_gamma, bn_beta)

    nc = _build(nonzero_b, nonzero_convb)
    if not nc.is_finalized():
        nc.finalize()   # Bacc: runs the wait-splitting/regalloc compile passes
    LAST_NC = nc

    in_maps = []
    for c in range(NCORES):
        in_maps.append(dict(
            ahat=np.ascontiguousarray(bd[c]),
            xp=np.ascontiguousarray(xp[c]),
            cw=cw,
            chd=chd,
        ))

    LAST = run_bass_kernel_spmd(nc, in_maps, core_ids=list(range(NCORES)),
                                trace=TRACE)
    outs = [LAST.results[c]["out"] for c in range(NCORES)]
    return np.concatenate(outs, axis=0).reshape(BS, T, NCLS)


# revision 54
# speedup vs baseline: 1.0027x; 1.0027x over previous
"""Trainium2 Bass kernel for nn_BaseContextAwareModel (4-layer GCN + mean-pool + conv1d head).

Strategy (per the graph-id sharding hint): 240 of the 1920 independent 22-node
frame-graphs per NeuronCore (= 2 clips/core), 5 graphs packed per 110-node
block-diagonal chunk. Host precomputes the GCN-normalized dense adjacency
(D^-1/2 (A+I) D^-1/2, transposed, block-diag packed) and folds x @ W1.

Performance design (sim-guided; 30.6us vs 37.9us baseline):
- fp8e4 shipping for ahat (at 8x scale; relu commutes with positive scale so
  1/8 folds into W2..W4 and the conv weights) and for XW1: halves the
  startup-critical DMA bytes. GCN compute stays bf16 with f32 PSUM.
- Input DMAs split across the SP/HWDGE queue AND the Pool/SWDGE queue (which
  bypasses the serialized HWDGE device), sliced so layer 0/1 start while
  later slices stream; head constants ride last on SP.
- Per GCN layer: mm1 (M1t = H_c^T AhatT_c) col-stacks 4 (l3: 2) chunks at
  32-aligned PSUM partition bases via tile_position, 4 (3) more side-by-side
  in the free dim -> ONE wide PSUM->SBUF eviction per 16 (6) chunks. mm2
  (H' = M1t^T W + b, relu fused into eviction) reads the stacked m1 at row
  tile positions and is emitted in band PAIRS, each band writing its own
  bank of a 2-bank PSUM tile: concurrent row tiles must NEVER share a PSUM
  bank (hardware crashes; col tiles may share). Evictions are balanced
  across ACT/DVE by a running-cost model; software pipelining (per-layer
  mm1/mm2 skew) hides eviction latency.
- Head per clip (emitted for clip 0 mid-layer-3): pooledT via matmul with a
  0/1 pool matrix (1/22 folded into conv weights), conv1d(k=3) as shifted
  lhsT matmuls, then sigmoid/capsule-length fused via
  sigmoid(z)-0.5 = tanh(z/2)/2: ACT Tanh (BN-eval scale/2, shift/2), DVE
  square + grouped reduce, one ACT Sqrt(x/16) over both clips (exactly one
  act-table switch), single merged t-major output DMA.
"""

import os
from contextlib import ExitStack

import numpy as np

import concourse.bass as bass
import concourse.bacc as bacc
import concourse.tile as tile
from concourse import mybir
from concourse.bass_utils import run_bass_kernel_spmd

# ---- problem constants (hardcoded; kernel.py must be self-contained) ----
BS, T, P, G = 16, 120, 22, 1920
NCORES = 8
GPC = G // NCORES          # 240 graphs per core
CPG = 5                    # graphs per 128-partition chunk
CH = CPG * P               # 110 nodes per chunk
NCHUNK = GPC // CPG        # 48 chunks per core
BPC = BS // NCORES         # 2 batch items (clips) per core
KPB = T // CPG             # 24 chunks per clip
C_IN = 14
CHS = [16, 32, 64, 152]
DIMS = [C_IN] + CHS
NCLS, DIM_CAP = 17, 16
C_CONV = DIM_CAP * NCLS    # 272
BN_EPS = 1e-3

# layer-0 DMA slices (ahat arrives in these chunk ranges; front-loaded so
# compute can start early, and aligned so l1's first 16-chunk group is
# covered by the first two slices)
L0G = [(0, 5), (5, 16), (16, 28), (28, 38), (38, 48)]

# constsW column layout (bf16): replicated W2/W3/W4, poolm, ones, biases
CW_W2, CW_W3, CW_W4 = 0, 32, 96
CW_POOL, CW_ONES = 248, 253
CW_B1, CW_B2, CW_B3, CW_B4 = 381, 397, 429, 493
CW_W = 645
# constsH column layout (bf16): wc1, wc2, convb, bn(scale/2, shift/2)
CHD_WC1, CHD_WC2, CHD_CB, CHD_BN = 0, 816, 1632, 1904
CHD_W = 1906

F32 = mybir.dt.float32
BF16 = mybir.dt.bfloat16
FP8 = mybir.dt.float8e4
NPBF16 = np.dtype(mybir.dt.np(BF16))
NPFP8 = np.dtype(mybir.dt.np(FP8))
AHAT_SCALE = 8.0  # ahat shipped as fp8e4 at 8x scale; 1/8 folded into W2..W4/wc


TRACE = os.environ.get("KTRACE", "0") == "1"
LAST = None  # last BassKernelResults, for test harness introspection
LAST_NC = None  # last built bass.Bass module, for cost-model simulation


def _host_prep(x, edge_index, edge_attr, W1):
    """Dense normalized adjacency + per-core packed operands."""
    src = np.asarray(edge_index[0], np.int64)
    dst = np.asarray(edge_index[1], np.int64)
    w = np.asarray(edge_attr[:, 4], np.float32)

    A = np.zeros((G, P, P), np.float32)
    np.add.at(A, (dst // P, dst % P, src % P), w)
    deg = A.sum(axis=2) + 1.0                      # + self-loop weight 1
    dinv = 1.0 / np.sqrt(deg)                      # deg >= 1 always
    Ahat = dinv[:, :, None] * A * dinv[:, None, :]
    ii = np.arange(P)
    Ahat[:, ii, ii] += dinv * dinv                 # self loop: dinv[d]^2
    AhatT = np.ascontiguousarray(Ahat.transpose(0, 2, 1))  # [g, s, d]

    # block-diag pack: (NCORES, CH, NCHUNK*CH); rows = source node in chunk,
    # cols = chunk*CH + dest node in chunk
    bd = np.zeros((NCORES, CH, NCHUNK * CH), np.float32)
    bdv = bd.reshape(NCORES, CH, NCHUNK, CH)
    Ar = AhatT.reshape(NCORES, NCHUNK, CPG, P, P)
    for j in range(CPG):
        bdv[:, j * P:(j + 1) * P, :, j * P:(j + 1) * P] = \
            Ar[:, :, j].transpose(0, 2, 1, 3)
    bd *= AHAT_SCALE

    # layer-1 W folded on host: ship XW1 = x @ W1, packed (8, 110, 48, 16)
    xw = np.asarray(x, np.float32) @ np.asarray(W1, np.float32)
    xr = xw.reshape(NCORES, NCHUNK, CH, CHS[0])
    xp = np.ascontiguousarray(xr.transpose(0, 2, 1, 3))
    return bd.astype(NPFP8), xp.astype(NPFP8)


def _pack_consts(W2, W3, W4, bs, conv_w, conv_b, bn_gamma, bn_beta):
    """constsW [128, CW_W] and constsH [128, CHD_W], both bf16."""
    cw = np.zeros((128, CW_W), np.float32)
    for j in range(4):
        cw[32 * j:32 * j + 16, CW_W2:CW_W2 + 32] = W2 / AHAT_SCALE
        cw[32 * j:32 * j + 32, CW_W3:CW_W3 + 64] = W3 / AHAT_SCALE
    for j in range(2):
        cw[64 * j:64 * j + 64, CW_W4:CW_W4 + 152] = W4 / AHAT_SCALE
    for j in range(CPG):
        cw[j * P:(j + 1) * P, CW_POOL + j] = 1.0
    cw[0, CW_ONES:CW_ONES + 128] = 1.0
    cw[0, CW_B1:CW_B1 + 16] = bs[0] * AHAT_SCALE
    cw[0, CW_B2:CW_B2 + 32] = bs[1] * AHAT_SCALE
    cw[0, CW_B3:CW_B3 + 64] = bs[2] * AHAT_SCALE
    cw[0, CW_B4:CW_B4 + 152] = bs[3] * AHAT_SCALE

    ch = np.zeros((128, CHD_W), np.float32)
    # conv weights (co, ci, k) -> (ci, k*272), with the 1/22 mean-pool factor
    wct = np.asarray(conv_w, np.float32).transpose(1, 2, 0) / (float(P) * AHAT_SCALE)
    ch[:, CHD_WC1:CHD_WC1 + 816] = wct[:128].reshape(128, 816)
    ch[:24, CHD_WC2:CHD_WC2 + 816] = wct[128:].reshape(24, 816)
    ch[0, CHD_CB:CHD_CB + C_CONV] = np.asarray(conv_b, np.float32)
    scale = np.asarray(bn_gamma, np.float32) / np.sqrt(1.0 + BN_EPS)
    ch[:T, CHD_BN] = scale * 0.5
    ch[:T, CHD_BN + 1] = np.asarray(bn_beta, np.float32) * 0.5
    return cw.astype(NPBF16), ch.astype(NPBF16)


def _build(nonzero_b, nonzero_convb):
    """Build the SPMD Bass program (identical on all 8 cores)."""
    nc = bacc.Bacc()
    AF = mybir.ActivationFunctionType

    d_ahat = nc.declare_dram_parameter("ahat", [CH, NCHUNK * CH], FP8, isOutput=False)
    d_x = nc.declare_dram_parameter("xp", [CH, NCHUNK, CHS[0]], FP8, isOutput=False)
    d_cw = nc.declare_dram_parameter("cw", [128, CW_W], BF16, isOutput=False)
    d_ch = nc.declare_dram_parameter("chd", [128, CHD_W], BF16, isOutput=False)
    d_out = nc.declare_dram_parameter("out", [BPC * T, NCLS], F32, isOutput=True)

    with tile.TileContext(nc) as tc, ExitStack() as ctx:
        const = ctx.enter_context(tc.tile_pool(name="const", bufs=1))
        state = ctx.enter_context(tc.tile_pool(name="state", bufs=1))
        m1p = ctx.enter_context(tc.tile_pool(name="m1sb", bufs=4))
        ps = ctx.enter_context(tc.tile_pool(name="ps", bufs=2, space="PSUM"))
        head = ctx.enter_context(tc.tile_pool(name="head", bufs=2))

        # ---- input DMAs ----
        # SP engine (HWDGE): ahat+xw interleaved so layer 0 starts ASAP.
        t_ahat_sl = []
        for i, (a, b) in enumerate(L0G):
            ta = const.tile([CH, (b - a) * CH], FP8, tag=f"ahat{i}", name=f"ta{i}")
            t_ahat_sl.append(ta)
        t_x = const.tile([CH, NCHUNK, CHS[0]], FP8)
        # dispatch is split across two queues (SP/HWDGE and Pool/SWDGE) so
        # the ~0.6-1us per-DMA dispatch overheads run in parallel
        nc.sync.dma_start(out=t_ahat_sl[0], in_=d_ahat[:, 0:5 * CH])
        nc.sync.dma_start(out=t_x[:, 0:16, :], in_=d_x[:, 0:16, :])
        nc.sync.dma_start(out=t_ahat_sl[2], in_=d_ahat[:, 16 * CH:28 * CH])
        nc.sync.dma_start(out=t_x[:, 16:48, :], in_=d_x[:, 16:48, :])
        nc.sync.dma_start(out=t_ahat_sl[4], in_=d_ahat[:, 38 * CH:48 * CH])
        # head constants ride the SP queue last: their transfer must not
        # delay the compute-critical ahat stream
        t_ch = const.tile([128, CHD_W], BF16)
        nc.sync.dma_start(out=t_ch, in_=d_ch[:])
        nc.gpsimd.dma_start(out=t_ahat_sl[1], in_=d_ahat[:, 5 * CH:16 * CH])
        t_cw = const.tile([128, CW_W], BF16)
        nc.gpsimd.dma_start(out=t_cw, in_=d_cw[:])
        nc.gpsimd.dma_start(out=t_ahat_sl[3], in_=d_ahat[:, 28 * CH:38 * CH])

        def ahat_chunk(k):
            for i, (a, b) in enumerate(L0G):
                if a <= k < b:
                    return t_ahat_sl[i][:, (k - a) * CH:(k - a + 1) * CH]
            raise AssertionError(k)

        # eviction engine selection: running-debt balance between ACT
        # (0.833 ns/elem + ~143 ns) and DVE (1.042 ns/elem + ~125 ns)
        debt = {"act": 0.0, "dve": 0.0}  # ACT also pays the tail table load

        def evict(dst, src, relu, eng=None):
            n = src.free_size()
            c_act, c_dve = n * 0.833 + 143.0, n * 1.042 + 125.0
            use_act = (eng == "act") if eng else (
                debt["act"] + c_act <= debt["dve"] + c_dve)
            if use_act:
                debt["act"] += c_act
                nc.scalar.activation(dst, src, AF.Relu if relu else AF.Copy)
            else:
                debt["dve"] += c_dve
                if relu:
                    nc.vector.tensor_scalar_max(dst, src, 0.0)
                else:
                    nc.vector.tensor_copy(dst, src)

        # ---- layer 0: H1 = relu(Ahat @ XW1 + b1), chunk groups = DMA slices
        # h1 carries 16 zero pad channels so l1's mm1 (lhsT free dim 32)
        # fully writes its 32-row PSUM bands (no uninitialized PSUM reads)
        t_ones = t_cw[0:1, CW_ONES:CW_ONES + 128]
        h1 = state.tile([CH, NCHUNK, 32], BF16)
        nc.gpsimd.memset(h1[:, :, CHS[0]:32], 0.0)
        for g, (a, b) in enumerate(L0G):
            ps_h = ps.tile([128, 512], F32, tag="m1", name="ps0", bufs=3)
            for k in range(a, b):
                j = k - a
                nc.tensor.matmul(
                    ps_h[:CH, j * 16:(j + 1) * 16],
                    lhsT=ahat_chunk(k), rhs=t_x[:, k, :],
                    start=True, stop=not nonzero_b[0])
                if nonzero_b[0]:
                    nc.tensor.matmul(
                        ps_h[:CH, j * 16:(j + 1) * 16],
                        lhsT=t_ones[:, :CH], rhs=t_cw[0:1, CW_B1:CW_B1 + 16],
                        start=False, stop=True, skip_group_check=True)
            evict(h1[:, a:b, 0:CHS[0]], ps_h[:CH, :(b - a) * 16], relu=True)

        # ---- head emission (called per clip; clip 0 interleaved into l3) ----
        bn_done = [False]
        t_bn = const.tile([T, 2], F32)
        ssum = head.tile([T, BPC * NCLS], F32, tag="ssum", name="ssum", bufs=1)

        def emit_head(b, h4):
            if not bn_done[0]:
                # bn scale/shift cast to f32 for the ACT bias/scale operands
                # (emitted late so DVE's in-order queue doesn't stall on the
                # chd DMA early on)
                nc.vector.tensor_copy(t_bn, t_ch[0:T, CHD_BN:CHD_BN + 2])
                bn_done[0] = True
            t_pool = t_cw[0:CH, CW_POOL:CW_POOL + CPG]
            # pooledT: pt1 (128ch) at psum cols 0:120, pt2 (24ch) at 128:248
            ps_pt = ps.tile([128, 512], F32, tag="head", name="pspt", bufs=1)
            for kk in range(KPB):
                k = b * KPB + kk
                nc.tensor.matmul(ps_pt[0:128, kk * CPG:(kk + 1) * CPG],
                                 lhsT=h4[:, k, :128], rhs=t_pool,
                                 start=True, stop=True)
                nc.tensor.matmul(ps_pt[0:24, 128 + kk * CPG:128 + (kk + 1) * CPG],
                                 lhsT=h4[:, k, 128:], rhs=t_pool,
                                 start=True, stop=True)
            # pt sbuf: [128, 244] = two 122-col blocks (1-col zero pad each side)
            t_pt = head.tile([128, 244], BF16, tag="pt", name="tpt")
            ptv = t_pt.rearrange("p (b c) -> p b c", b=2)
            nc.gpsimd.memset(ptv[:, :, 0:1], 0.0)
            nc.gpsimd.memset(ptv[:, :, 121:122], 0.0)
            evict(ptv[0:128, 0, 1:121], ps_pt[0:128, 0:120], relu=False)
            evict(ptv[0:24, 1, 1:121], ps_pt[0:24, 128:248], relu=False)

            # conv1d(k=3): 6 accumulating matmuls into one (120, 272) bank
            ps_caps = ps.tile([128, 512], F32, tag="head", name="pscaps", bufs=1)
            nmm = 6 + (1 if nonzero_convb else 0)
            i = 0
            for blk, p0, n in ((0, 0, 128), (1, 0, 24)):
                for kk in range(3):
                    wcols = (CHD_WC1 if blk == 0 else CHD_WC2) + kk * C_CONV
                    nc.tensor.matmul(
                        ps_caps[0:T, 0:C_CONV],
                        lhsT=t_pt[p0:p0 + n, blk * 122 + kk:blk * 122 + kk + T],
                        rhs=t_ch[p0:p0 + n, wcols:wcols + C_CONV],
                        start=(i == 0), stop=(i == nmm - 1),
                        skip_group_check=True)
                    i += 1
            if nonzero_convb:
                nc.tensor.matmul(ps_caps[0:T, 0:C_CONV], lhsT=t_ones[:, :T],
                                 rhs=t_ch[0:1, CHD_CB:CHD_CB + C_CONV],
                                 start=False, stop=True, skip_group_check=True)

            # sigmoid(z)-0.5 = tanh(z/2)/2; capsule length via square+reduce
            th = head.tile([T, C_CONV], BF16, tag="th", name="th")
            nc.scalar.activation(th, ps_caps[0:T, 0:C_CONV], AF.Tanh,
                                 bias=t_bn[:, 1:2], scale=t_bn[:, 0:1])
            debt["act"] += 370.0
            sq = head.tile([T, C_CONV], BF16, tag="sq", name="sq")
            nc.vector.tensor_mul(sq, th, th)
            nc.vector.reduce_sum(
                out=ssum[:, b * NCLS:(b + 1) * NCLS],
                in_=sq.rearrange("p (d c) -> p c d", c=NCLS),
                axis=mybir.AxisListType.X)

        # ---- layers 1..3 ----
        # mm1 col-stacks nband chunks per PSUM bank (tile_position col =
        # band*step) -> one wide m1 eviction per group. mm2 runs at row
        # tile_position band*step and is emitted in band PAIRS, each pair
        # writing its own bank of a 2-bank "hpair" tile: concurrent row
        # tiles must never share a PSUM bank (HW crashes), col tiles may.
        # per-layer: (cin, cin_load, cout, step, nband, wins, W col, b col)
        LSPEC = [
            (16, 32, 32, 32, 4, 4, CW_W2, CW_B2),
            (32, 32, 64, 32, 4, 4, CW_W3, CW_B3),
            (64, 64, 152, 64, 2, 3, CW_W4, CW_B4),
        ]
        h_prev = h1
        prev_tail = []
        for li, (cin, cin_load, cout, step, nband, wins, wcol, bcol) in enumerate(LSPEC):
            l = li + 1
            grp = nband * wins           # chunks per mm1 PSUM bank
            ngrp = NCHUNK // grp
            npair = nband // 2           # mm2 band-pairs per mm1 group
            nb_rows = (nband - 1) * step + cin_load
            h_next = state.tile([CH, NCHUNK, cout], BF16, tag=f"h{l}", name=f"h{l}")

            m1ref = {}

            def mm1_group(g, cin_load=cin_load, step=step, wins=wins,
                          grp=grp, nb_rows=nb_rows):
                ps_m1 = ps.tile([128, 512], F32, tag="m1", name="psm1", bufs=3)
                for j in range(grp):
                    k = g * grp + j
                    band, w = j // wins, j % wins
                    nc.tensor.matmul(
                        ps_m1[band * step:band * step + cin_load, w * CH:(w + 1) * CH],
                        lhsT=h_prev[:, k, :cin_load], rhs=ahat_chunk(k),
                        start=True, stop=True,
                        tile_position=(0, band * step))
                m1_sb = m1p.tile([128, 440], BF16, tag="m1sb", name="m1sb")
                evict(m1_sb[:nb_rows, :wins * CH], ps_m1[:nb_rows, :wins * CH],
                      relu=False)
                for j in range(grp):
                    band, w = j // wins, j % wins
                    m1ref[g * grp + j] = (m1_sb, band * step, w * CH)

            def mm2_pair(g, pr, cin=cin, cout=cout, step=step, wins=wins,
                         grp=grp, wcol=wcol, bcol=bcol, l=l, m1ref=m1ref,
                         h_next=h_next):
                # bands 2*pr and 2*pr+1 -> banks 0 and 1 of this pair tile
                ps_h = ps.tile([128, 1024], F32, tag="hpair", name="psh", bufs=2)
                for jj in range(2 * wins):
                    half, w = jj // wins, jj % wins
                    band = 2 * pr + half
                    sb, pb, co = m1ref[g * grp + band * wins + w]
                    dst_c = half * 512 + w * cout
                    nc.tensor.matmul(
                        ps_h[:CH, dst_c:dst_c + cout],
                        lhsT=sb[pb:pb + cin, co:co + CH],
                        rhs=t_cw[pb:pb + cin, wcol:wcol + cout],
                        start=True, stop=not nonzero_b[l],
                        tile_position=(pb, 0))
                    if nonzero_b[l]:
                        nc.tensor.matmul(
                            ps_h[:CH, dst_c:dst_c + cout],
                            lhsT=t_ones[:, :CH], rhs=t_cw[0:1, bcol:bcol + cout],
                            start=False, stop=True, skip_group_check=True)
                # one eviction: dst chunk order (half, w, c) matches src
                c0 = g * grp + 2 * pr * wins
                dst = h_next[:, c0:c0 + 2 * wins, :]
                src = ps_h[:CH].rearrange("p (b c) -> p b c", b=2)[:, :, :wins * cout]
                evict(dst, src, relu=True)
                return c0 + 2 * wins     # chunks completed so far

            # software pipeline: mm1 emitted ahead of mm2; for the last
            # layer, clip 0's head is emitted as soon as its chunks are done
            last = li == len(LSPEC) - 1

            def mm2_step(state_, h_next=h_next, last=last, npair=npair,
                         mm2_pair=mm2_pair):
                g, pr = state_
                hi = mm2_pair(g, pr)
                if last and hi == 6 * int(os.environ.get("KHD", "4")):
                    emit_head(0, h_next)
                pr += 1
                return (g + 1, 0) if pr == npair else (g, pr)

            SKEW = int(os.environ.get("KSKEW", "212")[li])
            cur = (0, 0)
            for g in range(ngrp):
                mm1_group(g)
                if g == 0:
                    # finish the previous layer's deferred mm2 pairs here so
                    # they hide behind this layer's first mm1 group
                    for f in prev_tail:
                        f()
                    prev_tail = []
                while cur[0] <= g - SKEW:
                    cur = mm2_step(cur)
            # defer the trailing pairs into the next layer's emission window
            ndef = 0 if last else int(os.environ.get("KDEF", "0"))
            left = []
            while cur[0] < ngrp:
                left.append(cur)
                cur = (cur[0] + 1, 0) if cur[1] + 1 == npair else (cur[0], cur[1] + 1)
            for g_, pr_ in left[:len(left) - ndef]:
                mm2_step((g_, pr_))
            for g_, pr_ in left[len(left) - ndef:]:
                prev_tail.append(lambda g_=g_, pr_=pr_, f=mm2_step: f((g_, pr_)))
            h_prev = h_next

        # ---- head for clip 1 (clip 0 was interleaved into layer 3) ----
        emit_head(1, h_prev)
        # one Sqrt op over both clips -> exactly one act-table switch
        y = head.tile([T, BPC * NCLS], F32, tag="y", name="y", bufs=1)
        nc.scalar.activation(y, ssum, AF.Sqrt, scale=1.0 / DIM_CAP)
        nc.sync.dma_start(
            out=d_out.rearrange("(b t) c -> t b c", b=BPC),
            in_=y.rearrange("p (b c) -> p b c", b=BPC))

    return nc


def kernel(x, edge_index, batch, edge_attr, W1, b1, W2, b2, W3, b3, W4, b4,
           conv_w, conv_b, bn_gamma, bn_beta):
    global LAST, LAST_NC
    bd, xp = _host_prep(x, edge_index, edge_attr, W1)

    bs = [np.asarray(b_, np.float32) for b_ in (b1, b2, b3, b4)]
    nonzero_b = [bool(np.any(b_)) for b_ in bs]
    nonzero_convb = bool(np.any(np.asarray(conv_b, np.float32)))
    cw, chd = _pack_consts(np.asarray(W2, np.float32), np.asarray(W3, np.float32),
                           np.asarray(W4, np.float32), bs, conv_w, conv_b,
                           bn_gamma, bn_beta)

    nc = _build(nonzero_b, nonzero_convb)
    if not nc.is_finalized():
        nc.finalize()   # Bacc: runs the wait-splitting/regalloc compile passes
    LAST_NC = nc

    in_maps = []
    for c in range(NCORES):
        in_maps.append(dict(
            ahat=np.ascontiguousarray(bd[c]),
            xp=np.ascontiguousarray(xp[c]),
            cw=cw,
            chd=chd,
        ))

    LAST = run_bass_kernel_spmd(nc, in_maps, core_ids=list(range(NCORES)),
                                trace=TRACE)
    outs = [LAST.results[c]["out"] for c in range(NCORES)]
    return np.concatenate(outs, axis=0).reshape(BS, T, NCLS)


# revision 60
# speedup vs baseline: 1.0040x; 1.0014x over previous
"""Trainium2 Bass kernel for nn_BaseContextAwareModel (4-layer GCN + mean-pool + conv1d head).

Strategy (per the graph-id sharding hint): 240 of the 1920 independent 22-node
frame-graphs per NeuronCore (= 2 clips/core), 5 graphs packed per 110-node
block-diagonal chunk. Host precomputes the GCN-normalized dense adjacency
(D^-1/2 (A+I) D^-1/2, transposed, block-diag packed) and folds x @ W1.

Performance design (sim-guided; 30.6us vs 37.9us baseline):
- fp8e4 shipping for ahat (at 8x scale; relu commutes with positive scale so
  1/8 folds into W2..W4 and the conv weights) and for XW1: halves the
  startup-critical DMA bytes. GCN compute stays bf16 with f32 PSUM.
- Input DMAs split across the SP/HWDGE queue AND the Pool/SWDGE queue (which
  bypasses the serialized HWDGE device), sliced so layer 0/1 start while
  later slices stream; head constants ride last on SP.
- Per GCN layer: mm1 (M1t = H_c^T AhatT_c) col-stacks 4 (l3: 2) chunks at
  32-aligned PSUM partition bases via tile_position, 4 (3) more side-by-side
  in the free dim -> ONE wide PSUM->SBUF eviction per 16 (6) chunks. mm2
  (H' = M1t^T W + b, relu fused into eviction) reads the stacked m1 at row
  tile positions and is emitted in band PAIRS, each band writing its own
  bank of a 2-bank PSUM tile: concurrent row tiles must NEVER share a PSUM
  bank (hardware crashes; col tiles may share). Evictions are balanced
  across ACT/DVE by a running-cost model; software pipelining (per-layer
  mm1/mm2 skew) hides eviction latency.
- Head per clip (emitted for clip 0 mid-layer-3): pooledT via matmul with a
  0/1 pool matrix (1/22 folded into conv weights), conv1d(k=3) as shifted
  lhsT matmuls, then sigmoid/capsule-length fused via
  sigmoid(z)-0.5 = tanh(z/2)/2: ACT Tanh (BN-eval scale/2, shift/2), DVE
  square + grouped reduce, one ACT Sqrt(x/16) over both clips (exactly one
  act-table switch), single merged t-major output DMA.
"""

import os
from contextlib import ExitStack

import numpy as np

import concourse.bass as bass
import concourse.bacc as bacc
import concourse.tile as tile
from concourse import mybir
from concourse.bass_utils import run_bass_kernel_spmd

# ---- problem constants (hardcoded; kernel.py must be self-contained) ----
BS, T, P, G = 16, 120, 22, 1920
NCORES = 8
GPC = G // NCORES          # 240 graphs per core
CPG = 5                    # graphs per 128-partition chunk
CH = CPG * P               # 110 nodes per chunk
NCHUNK = GPC // CPG        # 48 chunks per core
BPC = BS // NCORES         # 2 batch items (clips) per core
KPB = T // CPG             # 24 chunks per clip
C_IN = 14
CHS = [16, 32, 64, 152]
DIMS = [C_IN] + CHS
NCLS, DIM_CAP = 17, 16
C_CONV = DIM_CAP * NCLS    # 272
BN_EPS = 1e-3

# layer-0 DMA slices (ahat arrives in these chunk ranges; front-loaded so
# compute can start early, and aligned so l1's first 16-chunk group is
# covered by the first two slices)
L0G = [(0, 5), (5, 16), (16, 28), (28, 38), (38, 48)]

# constsW column layout (bf16): replicated W2/W3/W4, poolm, ones, biases
CW_W2, CW_W3, CW_W4 = 0, 32, 96
CW_POOL, CW_ONES = 248, 253
CW_B1, CW_B2, CW_B3, CW_B4 = 381, 397, 429, 493
CW_W = 645
# constsH column layout (bf16): wc1, wc2, convb, bn(scale/2, shift/2)
CHD_WC1, CHD_WC2, CHD_CB, CHD_BN = 0, 816, 1632, 1904
CHD_W = 1906

F32 = mybir.dt.float32
BF16 = mybir.dt.bfloat16
FP8 = mybir.dt.float8e4
NPBF16 = np.dtype(mybir.dt.np(BF16))
NPFP8 = np.dtype(mybir.dt.np(FP8))
AHAT_SCALE = 8.0  # ahat shipped as fp8e4 at 8x scale; 1/8 folded into W2..W4/wc


TRACE = os.environ.get("KTRACE", "0") == "1"
LAST = None  # last BassKernelResults, for test harness introspection
LAST_NC = None  # last built bass.Bass module, for cost-model simulation


def _host_prep(x, edge_index, edge_attr, W1):
    """Dense normalized adjacency + per-core packed operands."""
    src = np.asarray(edge_index[0], np.int64)
    dst = np.asarray(edge_index[1], np.int64)
    w = np.asarray(edge_attr[:, 4], np.float32)

    A = np.zeros((G, P, P), np.float32)
    np.add.at(A, (dst // P, dst % P, src % P), w)
    deg = A.sum(axis=2) + 1.0                      # + self-loop weight 1
    dinv = 1.0 / np.sqrt(deg)                      # deg >= 1 always
    Ahat = dinv[:, :, None] * A * dinv[:, None, :]
    ii = np.arange(P)
    Ahat[:, ii, ii] += dinv * dinv                 # self loop: dinv[d]^2
    AhatT = np.ascontiguousarray(Ahat.transpose(0, 2, 1))  # [g, s, d]

    # block-diag pack: (NCORES, CH, NCHUNK*CH); rows = source node in chunk,
    # cols = chunk*CH + dest node in chunk
    bd = np.zeros((NCORES, CH, NCHUNK * CH), np.float32)
    bdv = bd.reshape(NCORES, CH, NCHUNK, CH)
    Ar = AhatT.reshape(NCORES, NCHUNK, CPG, P, P)
    for j in range(CPG):
        bdv[:, j * P:(j + 1) * P, :, j * P:(j + 1) * P] = \
            Ar[:, :, j].transpose(0, 2, 1, 3)
    bd *= AHAT_SCALE

    # layer-1 W folded on host: ship XW1 = x @ W1, packed (8, 110, 48, 16)
    xw = np.asarray(x, np.float32) @ np.asarray(W1, np.float32)
    xr = xw.reshape(NCORES, NCHUNK, CH, CHS[0])
    xp = np.ascontiguousarray(xr.transpose(0, 2, 1, 3))
    return bd.astype(NPFP8), xp.astype(NPFP8)


def _pack_consts(W2, W3, W4, bs, conv_w, conv_b, bn_gamma, bn_beta):
    """constsW [128, CW_W] and constsH [128, CHD_W], both bf16."""
    cw = np.zeros((128, CW_W), np.float32)
    for j in range(4):
        cw[32 * j:32 * j + 16, CW_W2:CW_W2 + 32] = W2 / AHAT_SCALE
        cw[32 * j:32 * j + 32, CW_W3:CW_W3 + 64] = W3 / AHAT_SCALE
    for j in range(2):
        cw[64 * j:64 * j + 64, CW_W4:CW_W4 + 152] = W4 / AHAT_SCALE
    for j in range(CPG):
        cw[j * P:(j + 1) * P, CW_POOL + j] = 1.0
    cw[0, CW_ONES:CW_ONES + 128] = 1.0
    cw[0, CW_B1:CW_B1 + 16] = bs[0] * AHAT_SCALE
    cw[0, CW_B2:CW_B2 + 32] = bs[1] * AHAT_SCALE
    cw[0, CW_B3:CW_B3 + 64] = bs[2] * AHAT_SCALE
    cw[0, CW_B4:CW_B4 + 152] = bs[3] * AHAT_SCALE

    ch = np.zeros((128, CHD_W), np.float32)
    # conv weights (co, ci, k) -> (ci, k*272), with the 1/22 mean-pool factor
    wct = np.asarray(conv_w, np.float32).transpose(1, 2, 0) / (float(P) * AHAT_SCALE)
    ch[:, CHD_WC1:CHD_WC1 + 816] = wct[:128].reshape(128, 816)
    ch[:24, CHD_WC2:CHD_WC2 + 816] = wct[128:].reshape(24, 816)
    ch[0, CHD_CB:CHD_CB + C_CONV] = np.asarray(conv_b, np.float32)
    scale = np.asarray(bn_gamma, np.float32) / np.sqrt(1.0 + BN_EPS)
    ch[:T, CHD_BN] = scale * 0.5
    ch[:T, CHD_BN + 1] = np.asarray(bn_beta, np.float32) * 0.5
    return cw.astype(NPBF16), ch.astype(NPBF16)


def _build(nonzero_b, nonzero_convb):
    """Build the SPMD Bass program (identical on all 8 cores)."""
    nc = bacc.Bacc()
    AF = mybir.ActivationFunctionType

    d_ahat = nc.declare_dram_parameter("ahat", [CH, NCHUNK * CH], FP8, isOutput=False)
    d_x = nc.declare_dram_parameter("xp", [CH, NCHUNK, CHS[0]], FP8, isOutput=False)
    d_cw = nc.declare_dram_parameter("cw", [128, CW_W], BF16, isOutput=False)
    d_ch = nc.declare_dram_parameter("chd", [128, CHD_W], BF16, isOutput=False)
    d_out = nc.declare_dram_parameter("out", [BPC * T, NCLS], F32, isOutput=True)

    with tile.TileContext(nc) as tc, ExitStack() as ctx:
        const = ctx.enter_context(tc.tile_pool(name="const", bufs=1))
        state = ctx.enter_context(tc.tile_pool(name="state", bufs=1))
        m1p = ctx.enter_context(tc.tile_pool(name="m1sb", bufs=4))
        ps = ctx.enter_context(tc.tile_pool(name="ps", bufs=2, space="PSUM"))
        head = ctx.enter_context(tc.tile_pool(name="head", bufs=2))

        # ---- input DMAs ----
        # SP engine (HWDGE): ahat+xw interleaved so layer 0 starts ASAP.
        t_ahat_sl = []
        for i, (a, b) in enumerate(L0G):
            ta = const.tile([CH, (b - a) * CH], FP8, tag=f"ahat{i}", name=f"ta{i}")
            t_ahat_sl.append(ta)
        t_x = const.tile([CH, NCHUNK, CHS[0]], FP8)
        # dispatch is split across two queues (SP/HWDGE and Pool/SWDGE) so
        # the ~0.6-1us per-DMA dispatch overheads run in parallel
        nc.sync.dma_start(out=t_ahat_sl[0], in_=d_ahat[:, 0:5 * CH])
        nc.sync.dma_start(out=t_x[:, 0:16, :], in_=d_x[:, 0:16, :])
        nc.sync.dma_start(out=t_ahat_sl[2], in_=d_ahat[:, 16 * CH:28 * CH])
        nc.sync.dma_start(out=t_x[:, 16:48, :], in_=d_x[:, 16:48, :])
        nc.sync.dma_start(out=t_ahat_sl[4], in_=d_ahat[:, 38 * CH:48 * CH])
        # head constants ride the SP queue last: their transfer must not
        # delay the compute-critical ahat stream
        t_ch = const.tile([128, CHD_W], BF16)
        nc.sync.dma_start(out=t_ch, in_=d_ch[:])
        nc.gpsimd.dma_start(out=t_ahat_sl[1], in_=d_ahat[:, 5 * CH:16 * CH])
        t_cw = const.tile([128, CW_W], BF16)
        nc.gpsimd.dma_start(out=t_cw, in_=d_cw[:])
        nc.gpsimd.dma_start(out=t_ahat_sl[3], in_=d_ahat[:, 28 * CH:38 * CH])

        def ahat_chunk(k):
            for i, (a, b) in enumerate(L0G):
                if a <= k < b:
                    return t_ahat_sl[i][:, (k - a) * CH:(k - a + 1) * CH]
            raise AssertionError(k)

        # eviction engine selection: running-debt balance between ACT
        # (0.833 ns/elem + ~143 ns) and DVE (1.042 ns/elem + ~125 ns)
        debt = {"act": 0.0, "dve": 0.0}  # ACT also pays the tail table load

        def evict(dst, src, relu, eng=None):
            n = src.free_size()
            c_act, c_dve = n * 0.833 + 143.0, n * 1.042 + 125.0
            use_act = (eng == "act") if eng else (
                debt["act"] + c_act <= debt["dve"] + c_dve)
            if use_act:
                debt["act"] += c_act
                nc.scalar.activation(dst, src, AF.Relu if relu else AF.Copy)
            else:
                debt["dve"] += c_dve
                if relu:
                    nc.vector.tensor_scalar_max(dst, src, 0.0)
                else:
                    nc.vector.tensor_copy(dst, src)

        # ---- layer 0: H1 = relu(Ahat @ XW1 + b1), chunk groups = DMA slices
        # h1 carries 16 zero pad channels so l1's mm1 (lhsT free dim 32)
        # fully writes its 32-row PSUM bands (no uninitialized PSUM reads)
        t_ones = t_cw[0:1, CW_ONES:CW_ONES + 128]
        h1 = state.tile([CH, NCHUNK, 32], BF16)
        nc.gpsimd.memset(h1[:, :, CHS[0]:32], 0.0)
        for g, (a, b) in enumerate(L0G):
            ps_h = ps.tile([128, 512], F32, tag="m1", name="ps0", bufs=3)
            for k in range(a, b):
                j = k - a
                nc.tensor.matmul(
                    ps_h[:CH, j * 16:(j + 1) * 16],
                    lhsT=ahat_chunk(k), rhs=t_x[:, k, :],
                    start=True, stop=not nonzero_b[0])
                if nonzero_b[0]:
                    nc.tensor.matmul(
                        ps_h[:CH, j * 16:(j + 1) * 16],
                        lhsT=t_ones[:, :CH], rhs=t_cw[0:1, CW_B1:CW_B1 + 16],
                        start=False, stop=True, skip_group_check=True)
            evict(h1[:, a:b, 0:CHS[0]], ps_h[:CH, :(b - a) * 16], relu=True)

        # ---- head emission (called per clip; clip 0 interleaved into l3) ----
        bn_done = [False]
        t_bn = const.tile([T, 2], F32)
        ssum = head.tile([T, BPC * NCLS], F32, tag="ssum", name="ssum", bufs=1)

        def emit_head(b, h4):
            if not bn_done[0]:
                # bn scale/shift cast to f32 for the ACT bias/scale operands
                # (emitted late so DVE's in-order queue doesn't stall on the
                # chd DMA early on)
                nc.vector.tensor_copy(t_bn, t_ch[0:T, CHD_BN:CHD_BN + 2])
                bn_done[0] = True
            t_pool = t_cw[0:CH, CW_POOL:CW_POOL + CPG]
            # pooledT: pt1 (128ch) at psum cols 0:120, pt2 (24ch) at 128:248
            ps_pt = ps.tile([128, 512], F32, tag="head", name="pspt", bufs=1)
            for kk in range(KPB):
                k = b * KPB + kk
                nc.tensor.matmul(ps_pt[0:128, kk * CPG:(kk + 1) * CPG],
                                 lhsT=h4[:, k, :128], rhs=t_pool,
                                 start=True, stop=True)
                nc.tensor.matmul(ps_pt[0:24, 128 + kk * CPG:128 + (kk + 1) * CPG],
                                 lhsT=h4[:, k, 128:], rhs=t_pool,
                                 start=True, stop=True)
            # pt sbuf: [128, 244] = two 122-col blocks (1-col zero pad each side)
            t_pt = head.tile([128, 244], BF16, tag="pt", name="tpt")
            ptv = t_pt.rearrange("p (b c) -> p b c", b=2)
            nc.gpsimd.memset(ptv[:, :, 0:1], 0.0)
            nc.gpsimd.memset(ptv[:, :, 121:122], 0.0)
            evict(ptv[0:128, 0, 1:121], ps_pt[0:128, 0:120], relu=False)
            evict(ptv[0:24, 1, 1:121], ps_pt[0:24, 128:248], relu=False)

            # conv1d(k=3): 6 accumulating matmuls into one (120, 272) bank
            ps_caps = ps.tile([128, 512], F32, tag="head", name="pscaps", bufs=1)
            nmm = 6 + (1 if nonzero_convb else 0)
            i = 0
            for blk, p0, n in ((0, 0, 128), (1, 0, 24)):
                for kk in range(3):
                    wcols = (CHD_WC1 if blk == 0 else CHD_WC2) + kk * C_CONV
                    nc.tensor.matmul(
                        ps_caps[0:T, 0:C_CONV],
                        lhsT=t_pt[p0:p0 + n, blk * 122 + kk:blk * 122 + kk + T],
                        rhs=t_ch[p0:p0 + n, wcols:wcols + C_CONV],
                        start=(i == 0), stop=(i == nmm - 1),
                        skip_group_check=True)
                    i += 1
            if nonzero_convb:
                nc.tensor.matmul(ps_caps[0:T, 0:C_CONV], lhsT=t_ones[:, :T],
                                 rhs=t_ch[0:1, CHD_CB:CHD_CB + C_CONV],
                                 start=False, stop=True, skip_group_check=True)

            # sigmoid(z)-0.5 = tanh(z/2)/2; capsule length via square+reduce
            th = head.tile([T, C_CONV], BF16, tag="th", name="th")
            nc.scalar.activation(th, ps_caps[0:T, 0:C_CONV], AF.Tanh,
                                 bias=t_bn[:, 1:2], scale=t_bn[:, 0:1])
            debt["act"] += 370.0
            sq = head.tile([T, C_CONV], BF16, tag="sq", name="sq")
            nc.vector.tensor_mul(sq, th, th)
            nc.vector.reduce_sum(
                out=ssum[:, b * NCLS:(b + 1) * NCLS],
                in_=sq.rearrange("p (d c) -> p c d", c=NCLS),
                axis=mybir.AxisListType.X)

        # ---- layers 1..3 ----
        # mm1 col-stacks nband chunks per PSUM bank (tile_position col =
        # band*step) -> one wide m1 eviction per group. mm2 runs at row
        # tile_position band*step and is emitted in band PAIRS, each pair
        # writing its own bank of a 2-bank "hpair" tile: concurrent row
        # tiles must never share a PSUM bank (HW crashes), col tiles may.
        # per-layer: (cin, cin_load, cout, step, nband, wins, W col, b col)
        LSPEC = [
            (16, 32, 32, 32, 4, 4, CW_W2, CW_B2),
            (32, 32, 64, 32, 4, 4, CW_W3, CW_B3),
            (64, 64, 152, 64, 2, 3, CW_W4, CW_B4),
        ]
        h_prev = h1
        prev_tail = []
        for li, (cin, cin_load, cout, step, nband, wins, wcol, bcol) in enumerate(LSPEC):
            l = li + 1
            grp = nband * wins           # chunks per mm1 PSUM bank
            ngrp = NCHUNK // grp
            npair = nband // 2           # mm2 band-pairs per mm1 group
            nb_rows = (nband - 1) * step + cin_load
            h_next = state.tile([CH, NCHUNK, cout], BF16, tag=f"h{l}", name=f"h{l}")

            m1ref = {}

            def mm1_group(g, cin_load=cin_load, step=step, wins=wins,
                          grp=grp, nb_rows=nb_rows):
                ps_m1 = ps.tile([128, 512], F32, tag="m1", name="psm1", bufs=3)
                for j in range(grp):
                    k = g * grp + j
                    band, w = j // wins, j % wins
                    nc.tensor.matmul(
                        ps_m1[band * step:band * step + cin_load, w * CH:(w + 1) * CH],
                        lhsT=h_prev[:, k, :cin_load], rhs=ahat_chunk(k),
                        start=True, stop=True,
                        tile_position=(0, band * step))
                m1_sb = m1p.tile([128, 440], BF16, tag="m1sb", name="m1sb")
                evict(m1_sb[:nb_rows, :wins * CH], ps_m1[:nb_rows, :wins * CH],
                      relu=False)
                for j in range(grp):
                    band, w = j // wins, j % wins
                    m1ref[g * grp + j] = (m1_sb, band * step, w * CH)

            def mm2_pair(g, pr, cin=cin, cout=cout, step=step, wins=wins,
                         grp=grp, wcol=wcol, bcol=bcol, l=l, m1ref=m1ref,
                         h_next=h_next):
                # bands 2*pr and 2*pr+1 -> banks 0 and 1 of this pair tile
                ps_h = ps.tile([128, 1024], F32, tag="hpair", name="psh", bufs=2)
                for jj in range(2 * wins):
                    half, w = jj // wins, jj % wins
                    band = 2 * pr + half
                    sb, pb, co = m1ref[g * grp + band * wins + w]
                    dst_c = half * 512 + w * cout
                    nc.tensor.matmul(
                        ps_h[:CH, dst_c:dst_c + cout],
                        lhsT=sb[pb:pb + cin, co:co + CH],
                        rhs=t_cw[pb:pb + cin, wcol:wcol + cout],
                        start=True, stop=not nonzero_b[l],
                        tile_position=(pb, 0))
                    if nonzero_b[l]:
                        nc.tensor.matmul(
                            ps_h[:CH, dst_c:dst_c + cout],
                            lhsT=t_ones[:, :CH], rhs=t_cw[0:1, bcol:bcol + cout],
                            start=False, stop=True, skip_group_check=True)
                # one eviction: dst chunk order (half, w, c) matches src
                c0 = g * grp + 2 * pr * wins
                dst = h_next[:, c0:c0 + 2 * wins, :]
                src = ps_h[:CH].rearrange("p (b c) -> p b c", b=2)[:, :, :wins * cout]
                evict(dst, src, relu=True)
                return c0 + 2 * wins     # chunks completed so far

            # software pipeline: mm1 emitted ahead of mm2; for the last
            # layer, clip 0's head is emitted as soon as its chunks are done
            last = li == len(LSPEC) - 1

            def mm2_step(state_, h_next=h_next, last=last, npair=npair,
                         mm2_pair=mm2_pair):
                g, pr = state_
                hi = mm2_pair(g, pr)
                if last and hi == 6 * int(os.environ.get("KHD", "6")):
                    emit_head(0, h_next)
                pr += 1
                return (g + 1, 0) if pr == npair else (g, pr)

            SKEW = int(os.environ.get("KSKEW", "212")[li])
            cur = (0, 0)
            for g in range(ngrp):
                mm1_group(g)
                if g == 0:
                    # finish the previous layer's deferred mm2 pairs here so
                    # they hide behind this layer's first mm1 group
                    for f in prev_tail:
                        f()
                    prev_tail = []
                while cur[0] <= g - SKEW:
                    cur = mm2_step(cur)
            # defer the trailing pairs into the next layer's emission window
            ndef = 0 if last else int(os.environ.get("KDEF", "0"))
            left = []
            while cur[0] < ngrp:
                left.append(cur)
                cur = (cur[0] + 1, 0) if cur[1] + 1 == npair else (cur[0], cur[1] + 1)
            for g_, pr_ in left[:len(left) - ndef]:
                mm2_step((g_, pr_))
            for g_, pr_ in left[len(left) - ndef:]:
                prev_tail.append(lambda g_=g_, pr_=pr_, f=mm2_step: f((g_, pr_)))
            h_prev = h_next

        # ---- head for clip 1 (clip 0 was interleaved into layer 3) ----
        emit_head(1, h_prev)
        # one Sqrt op over both clips -> exactly one act-table switch
        y = head.tile([T, BPC * NCLS], F32, tag="y", name="y", bufs=1)
        nc.scalar.activation(y, ssum, AF.Sqrt, scale=1.0 / DIM_CAP)
        nc.sync.dma_start(
            out=d_out.rearrange("(b t) c -> t b c", b=BPC),
            in_=y.rearrange("p (b c) -> p b c", b=BPC))

    return nc


def kernel(x, edge_index, batch, edge_attr, W1, b1, W2, b2, W3, b3, W4, b4,
           conv_w, conv_b, bn_gamma, bn_beta):
    global LAST, LAST_NC
    bd, xp = _host_prep(x, edge_index, edge_attr, W1)

    bs = [np.asarray(b_, np.float32) for b_ in (b1, b2, b3, b4)]
    nonzero_b = [bool(np.any(b_)) for b_ in bs]
    nonzero_convb = bool(np.any(np.asarray(conv_b, np.float32)))
    cw, chd = _pack_consts(np.asarray(W2, np.float32), np.asarray(W3, np.float32),
                           np.asarray(W4, np.float32), bs, conv_w, conv_b,
                           bn_gamma, bn_beta)

    nc = _build(nonzero_b, nonzero_convb)
    if not nc.is_finalized():
        nc.finalize()   # Bacc: runs the wait-splitting/regalloc compile passes
    LAST_NC = nc

    in_maps = []
    for c in range(NCORES):
        in_maps.append(dict(
            ahat=np.ascontiguousarray(bd[c]),
            xp=np.ascontiguousarray(xp[c]),
            cw=cw,
            chd=chd,
        ))

    LAST = run_bass_kernel_spmd(nc, in_maps, core_ids=list(range(NCORES)),
                                trace=TRACE)
    outs = [LAST.results[c]["out"] for c in range(NCORES)]
    return np.concatenate(outs, axis=0).reshape(BS, T, NCLS)
